# revision 45
# baseline (speedup 1.0000x reference)
"""GATv2 2-layer GNN on 8 Trainium2 NeuronCores (self-contained).

Sharding: destination nodes (and their incident edges) are partitioned
across the 8 cores; weights replicated.  The host pre-permutes node
features into per-edge streaming order (halo exchange + gather done on
the host), so the device never does an indexed gather:

  - per edge-chunk of 128: u = x[src].T @ Wl + x[dst].T @ Wr accumulated
    in PSUM (two streaming matmuls; the per-edge operands arrive as
    plain sequential DMA).
  - logits: Prelu(u) on ScalarE, * att + per-head reduce on
    GpSimd/VectorE, Exp on ScalarE.
  - weighted sums: one-hot scatter matmul so += Q @ [ea | ea*u] where Q
    is a host-built 0/1 matrix (dst-in-block per edge).  Both the
    numerator sum_e ea*u and denominator sum_e ea accumulate in PSUM.
  - out[d] = (sum_e ea*u)/(sum_e ea) - xr[d]  (softmax weights sum to 1,
    so the xr[dst] part of u contributes exactly xr[d]; subtract it).
    xr = x_slice @ Wr is masked to 0 for edge-less nodes.
  - layernorm (+ELU for layer 1) runs in 4 batched end-passes over
    ~12-block segments, avoiding per-block scalar-engine table thrash
    (only Prelu/Exp/Copy/Sqrt are used).

The h1 exchange between the two layers is done on the host.
"""
import os
import sys
import numpy as np

sys.path.insert(0, "/opt/trn_rl_repo")

import ml_dtypes
import concourse.bacc as bacc
import concourse.mybir as mybir
from concourse.tile import TileContext
from concourse.bass_utils import run_bass_kernel_spmd

dt = mybir.dt
A = mybir.ActivationFunctionType
Op = mybir.AluOpType

N, E = 50000, 800000
F_IN, F_H, H1, F_OUT2 = 128, 16, 8, 64
F_OUT1 = H1 * F_H  # 128
NEG_SLOPE = 0.2
LN_EPS = 1e-5
N_CORES = 8
BLK = 128
S = N // N_CORES          # 6250 dst nodes per core
NB = 50                   # 49 live blocks + 1 pad block
NPAIR = NB // 2
SEG_PAIRS = (7, 6, 6, 6)  # end-pass segments (pairs)
G = 4                     # chunks per inner group
LAG = 4                   # groups of scatter-matmul deferral (sw pipeline)

# exec-time info from the most recent kernel() call (for test harnesses)
LAST_EXEC_NS = {}


# ---------------------------------------------------------------- host prep
def _host_prep(edge_index):
    """Edge layout shared by both layers: per core, edges sorted by dst,
    grouped into 128-dst blocks, chunked by 128 edges.  Returns per-core
    column->node permutations (src/dst), the scatter one-hot q, and the
    has-edge mask."""
    src = np.asarray(edge_index[0], dtype=np.int64)
    dst = np.asarray(edge_index[1], dtype=np.int64)

    order = np.argsort(dst, kind="stable")
    src_s, dst_s = src[order], dst[order]
    core_of = dst_s // S

    per_core = []
    counts = np.zeros((N_CORES, NB), dtype=np.int64)
    for c in range(N_CORES):
        m = core_of == c
        sc, dc = src_s[m], dst_s[m] - c * S
        b_of = dc // BLK
        counts[c] = np.bincount(b_of, minlength=NB)
        per_core.append((sc, dc, b_of))

    cblk = np.maximum(1, (counts.max(axis=0) + BLK - 1) // BLK)  # [NB]
    offC = np.concatenate([[0], np.cumsum(cblk)])
    C_total = int(offC[-1])

    cores = []
    for c in range(N_CORES):
        sc, dc, b_of = per_core[c]
        # edges are dst-sorted, so per-block runs are contiguous
        block_start = np.concatenate([[0], np.cumsum(counts[c])])
        j_in_block = np.arange(len(sc)) - block_start[b_of]
        col = (offC[b_of] + j_in_block // BLK) * BLK + j_in_block % BLK

        src_ids = np.zeros(C_total * BLK, dtype=np.int64)
        dst_ids = np.zeros(C_total * BLK, dtype=np.int64)
        src_ids[col] = sc
        dst_ids[col] = sc * 0 + (dc + c * S)
        q = np.zeros((BLK, C_total * BLK), dtype=ml_dtypes.bfloat16)
        lane = col % BLK
        chunk = col // BLK
        q.reshape(-1)[lane * (C_total * BLK) + chunk * BLK + (dc % BLK)] = 1.0
        dcol = np.full((BLK, C_total), -1.0, dtype=ml_dtypes.bfloat16)
        dcol[lane, chunk] = (dc % BLK).astype(np.float32)

        deg = np.bincount(dc, minlength=NB * BLK)[: NB * BLK]
        live = (np.arange(NB * BLK) < S) & (deg > 0)
        mask = np.ascontiguousarray(
            live.reshape(NB, BLK).T.astype(np.float32))  # [128, NB]
        cores.append(dict(src_ids=src_ids, dst_ids=dst_ids, q=q, dcol=dcol,
                          mask=mask))

    return dict(cblk=cblk, offC=offC, C_total=C_total, cores=cores)


def _perm_streams(meta, x_full, core):
    """Per-edge feature streams for one core: x[src].T and x[dst].T as
    [128, C_total*128] bf16."""
    xb = x_full if x_full.dtype == ml_dtypes.bfloat16 else \
        np.asarray(x_full, np.float32).astype(ml_dtypes.bfloat16)
    xts = np.ascontiguousarray(xb[core["src_ids"]].T)
    xtd = np.ascontiguousarray(xb[core["dst_ids"]].T)
    return xts, xtd


def _slice_stream(x_full, c):
    """Own dst-slice, transposed+padded to [128, NB*128] bf16 (for xr)."""
    sl = np.zeros((NB * BLK, x_full.shape[1]), dtype=np.float32)
    sl[:S] = np.asarray(x_full[c * S:(c + 1) * S], np.float32)
    return np.ascontiguousarray(sl.T).astype(ml_dtypes.bfloat16)


# ---------------------------------------------------------------- builder
def _build_layer(meta, F_out, H, layer, debug=False, split_P=None):
    """split_P: if not None, attention weights are host-folded into Wl/Wr
    (single-head only); columns [0:split_P] use Prelu alpha=0.2, the rest
    alpha=5.0, and the logits are a plain per-head sum of the Prelu output.
    The end-pass multiplies by the host-provided 1/s compensation."""
    cblk, offC, C_total = meta["cblk"], meta["offC"], meta["C_total"]
    C = F_out // H

    nc = bacc.Bacc("TRN2", target_bir_lowering=False, debug=False,
                   num_devices=N_CORES)
    xts_d = nc.dram_tensor("xts", [128, C_total * BLK], dt.bfloat16, kind="ExternalInput").ap()
    xtd_d = nc.dram_tensor("xtd", [128, C_total * BLK], dt.bfloat16, kind="ExternalInput").ap()
    dev_q = layer == 2  # build scatter one-hot on device (layer2 is DMA-bound)
    if dev_q:
        dcol_d = nc.dram_tensor("dcol", [128, C_total], dt.bfloat16, kind="ExternalInput").ap()
    else:
        q_d = nc.dram_tensor("q", [128, C_total * BLK], dt.bfloat16, kind="ExternalInput").ap()
    xTs = nc.dram_tensor("xTs", [128, NB * BLK], dt.bfloat16, kind="ExternalInput").ap()
    wl = nc.dram_tensor("wl", [128, F_out], dt.bfloat16, kind="ExternalInput").ap()
    wr = nc.dram_tensor("wr", [128, F_out], dt.bfloat16, kind="ExternalInput").ap()
    if split_P is not None:
        wrx_in = nc.dram_tensor("wrx", [128, F_out], dt.bfloat16, kind="ExternalInput").ap()
        sinv_in = nc.dram_tensor("sinv", [128, F_out], dt.float32, kind="ExternalInput").ap()
    else:
        att_in = nc.dram_tensor("att", [128, F_out], dt.bfloat16, kind="ExternalInput").ap()
    bias_in = nc.dram_tensor("bias", [128, F_out], dt.float32, kind="ExternalInput").ap()
    g_in = nc.dram_tensor("g", [128, F_out], dt.float32, kind="ExternalInput").ap()
    b_in = nc.dram_tensor("b", [128, F_out], dt.float32, kind="ExternalInput").ap()
    mask_in = nc.dram_tensor("mask", [128, NB], dt.float32, kind="ExternalInput").ap()
    hout = nc.dram_tensor("hout", [NB * BLK, F_out], dt.float32, kind="ExternalOutput").ap()
    if debug:
        dbg_u = nc.dram_tensor("dbg_u", [128, G * F_out], dt.float32, kind="ExternalOutput").ap()
        dbg_eav = nc.dram_tensor("dbg_eav", [128, G * (H + F_out)], dt.float32, kind="ExternalOutput").ap()
        dbg_xr = nc.dram_tensor("dbg_xr", [128, NB * F_out], dt.float32, kind="ExternalOutput").ap()
        dbg_sal = nc.dram_tensor("dbg_sal", [128, NPAIR * 2 * (H + F_out)], dt.float32, kind="ExternalOutput").ap()
        dbg_amul = nc.dram_tensor("dbg_amul", [128, G * F_out], dt.float32, kind="ExternalOutput").ap()

    with TileContext(nc) as tc:
        with (
            tc.tile_pool(name="con", bufs=1) as con,
            tc.tile_pool(name="st", bufs=3) as st,
            tc.tile_pool(name="ck", bufs=6) as ck,
            tc.tile_pool(name="ep", bufs=2) as ep,
            tc.tile_pool(name="ps_u", bufs=5, space="PSUM") as ps_u,
            tc.tile_pool(name="ps_acc", bufs=2, space="PSUM") as ps_acc,
        ):
            # constants
            wl_sb = con.tile([128, F_out], dt.bfloat16)
            nc.sync.dma_start(out=wl_sb[:], in_=wl[:])
            wr_sb = con.tile([128, F_out], dt.bfloat16)
            nc.sync.dma_start(out=wr_sb[:], in_=wr[:])
            if split_P is not None:
                wrx_sb = con.tile([128, F_out], dt.bfloat16)
                nc.sync.dma_start(out=wrx_sb[:], in_=wrx_in[:])
                sinv_sb = con.tile([128, F_out], dt.float32)
                nc.sync.dma_start(out=sinv_sb[:], in_=sinv_in[:])
            else:
                wrx_sb = wr_sb
                att_sb = con.tile([128, F_out], dt.bfloat16)
                nc.sync.dma_start(out=att_sb[:], in_=att_in[:])
            bias_sb = con.tile([128, F_out], dt.float32)
            nc.sync.dma_start(out=bias_sb[:], in_=bias_in[:])
            g_sb = con.tile([128, F_out], dt.float32)
            nc.sync.dma_start(out=g_sb[:], in_=g_in[:])
            b_sb = con.tile([128, F_out], dt.float32)
            nc.sync.dma_start(out=b_sb[:], in_=b_in[:])
            mask_sb = con.tile([128, NB], dt.float32)
            nc.sync.dma_start(out=mask_sb[:], in_=mask_in[:])
            sal = con.tile([128, NPAIR, 2, H + F_out], dt.float32)
            xr_sb = con.tile([128, NB, F_out], dt.float32)
            if dev_q:
                dcol_sb = con.tile([128, C_total], dt.bfloat16)
                nc.sync.dma_start(out=dcol_sb[:], in_=dcol_d[:])
                iota_row = con.tile([128, 128], dt.int32)
                nc.gpsimd.iota(iota_row[:], pattern=[[1, 128]], base=0,
                               channel_multiplier=0)
                iota_row_b = con.tile([128, 128], dt.bfloat16)
                nc.vector.tensor_copy(iota_row_b[:], iota_row[:])
                iota_bc1 = iota_row_b[:].rearrange("p (o f) -> p o f", o=1)

            # xr = x_slice @ Wr, masked to 0 for edge-less dst rows; then
            # xr_sb <- bias - xr so the end-pass needs one add, not two ops.
            ctx = nc.named_scope("xr"); ctx.__enter__()
            XB = G  # reuse the edge-phase PSUM tag/shape
            for t0 in range(0, NB, XB):
                n = min(XB, NB - t0)
                xs_t = st.tile([128, XB, 128], dt.bfloat16, tag="xs")
                nc.sync.dma_start(out=xs_t[:, :n, :],
                                  in_=xTs[:, t0 * 128:(t0 + n) * 128])
                pd = ps_u.tile([128, G, F_out], dt.float32, tag="ups")
                for i in range(n):
                    nc.tensor.matmul(pd[:, i, :], xs_t[:, i, :], wrx_sb[:],
                                     start=True, stop=True)
                    nc.scalar.activation(xr_sb[:, t0 + i, :], pd[:, i, :], A.Copy,
                                         scale=mask_sb[:, t0 + i:t0 + i + 1])
            nc.vector.scalar_tensor_tensor(
                xr_sb[:], xr_sb[:], -1.0,
                bias_sb[:].rearrange("p (o f) -> p o f", o=1)
                .to_broadcast([128, NB, F_out]),
                op0=Op.mult, op1=Op.add)
            ctx.__exit__(None, None, None)

            ctx = nc.named_scope("edge"); ctx.__enter__()
            if split_P is None:
                att_bc1 = att_sb[:].rearrange("p (o f) -> p o f", o=1)
            seg_pair_off = np.concatenate([[0], np.cumsum(SEG_PAIRS)])

            # deferred emission of scatter matmuls + pair drains: keeps the
            # in-order PE queue LAG groups ahead of the eav dependency
            fifo = []

            def _emit(item):
                if item[0] == "so":
                    for ps_ap, q_ap, eav_ap, st_, sp_ in item[1]:
                        nc.tensor.matmul(ps_ap, q_ap, eav_ap, start=st_, stop=sp_)
                else:
                    pair_, so_tile = item[1]
                    nc.scalar.activation(sal[:, pair_, :, :], so_tile[:], A.Copy)

            def _push(item):
                fifo.append(item)
                n_so = sum(1 for it in fifo if it[0] == "so")
                while n_so > LAG:
                    it = fifo.pop(0)
                    _emit(it)
                    if it[0] == "so":
                        n_so -= 1

            def _flush():
                while fifo:
                    _emit(fifo.pop(0))

            for seg in range(len(SEG_PAIRS)):
                for pair in range(seg_pair_off[seg], seg_pair_off[seg + 1]):
                    so_ps = ps_acc.tile([128, 2, H + F_out], dt.float32, tag="sops")
                    for jb in range(2):
                        b = 2 * pair + jb
                        cbk = int(cblk[b])
                        c0 = int(offC[b]) * BLK
                        xts_t = st.tile([128, cbk, 128], dt.bfloat16, tag="xts")
                        nc.sync.dma_start(out=xts_t[:], in_=xts_d[:, c0:c0 + cbk * BLK])
                        xtd_t = st.tile([128, cbk, 128], dt.bfloat16, tag="xtd")
                        nc.sync.dma_start(out=xtd_t[:], in_=xtd_d[:, c0:c0 + cbk * BLK])
                        if dev_q:
                            q_t = st.tile([128, cbk, 128], dt.bfloat16, tag="qt")
                            cc0 = int(offC[b])
                            nc.vector.tensor_tensor(
                                q_t[:],
                                iota_bc1.to_broadcast([128, cbk, 128]),
                                dcol_sb[:, cc0:cc0 + cbk]
                                .rearrange("p (k o) -> p k o", o=1)
                                .to_broadcast([128, cbk, 128]),
                                op=Op.is_equal)
                        else:
                            q_t = st.tile([128, cbk, 128], dt.bfloat16, tag="qt")
                            nc.sync.dma_start(out=q_t[:], in_=q_d[:, c0:c0 + cbk * BLK])

                        for k0 in range(0, cbk, G):
                            g = min(G, cbk - k0)
                            u_ps = ps_u.tile([128, G, F_out], dt.float32, tag="ups")
                            for j in range(g):
                                k = k0 + j
                                nc.tensor.matmul(u_ps[:, j, :], xts_t[:, k, :],
                                                 wl_sb[:], start=True, stop=False)
                                nc.tensor.matmul(u_ps[:, j, :], xtd_t[:, k, :],
                                                 wr_sb[:], start=False, stop=True)
                            lr = ck.tile([128, G, F_out], dt.bfloat16, tag="lr")
                            if split_P is not None:
                                if split_P > 0:
                                    nc.scalar.activation(
                                        lr[:, :g, 0:split_P],
                                        u_ps[:, :g, 0:split_P],
                                        A.Prelu, alpha=NEG_SLOPE)
                                if split_P < F_out:
                                    nc.scalar.activation(
                                        lr[:, :g, split_P:],
                                        u_ps[:, :g, split_P:],
                                        A.Prelu, alpha=1.0 / NEG_SLOPE)
                                amul = lr
                            else:
                                nc.scalar.activation(lr[:, :g, :], u_ps[:, :g, :],
                                                     A.Prelu, alpha=NEG_SLOPE)
                                amul = ck.tile([128, G, F_out], dt.bfloat16, tag="amul")
                                nc.gpsimd.tensor_tensor(
                                    amul[:, :g, :], lr[:, :g, :],
                                    att_bc1.to_broadcast([128, g, F_out]), op=Op.mult)
                            a4 = ck.tile([128, G, H], dt.float32, tag="a4")
                            nc.vector.tensor_reduce(
                                a4[:, :g, :],
                                amul[:, :g, :].rearrange("p g (h c) -> p g h c", h=H),
                                axis=mybir.AxisListType.X, op=Op.add)
                            eav = ck.tile([128, G, H + F_out], dt.bfloat16, tag="eav")
                            nc.scalar.activation(eav[:, :g, 0:H], a4[:, :g, :], A.Exp)
                            nc.vector.tensor_tensor(
                                eav[:, :g, H:].rearrange("p g (h c) -> p g h c", h=H),
                                u_ps[:, :g, :].rearrange("p g (h c) -> p g h c", h=H),
                                eav[:, :g, 0:H].rearrange("p g (h o) -> p g h o", o=1)
                                .to_broadcast([128, g, H, C]),
                                op=Op.mult)
                            if debug and b == 0 and k0 == 0:
                                _flush()
                                du = ck.tile([128, G, F_out], dt.float32, tag="du")
                                nc.vector.tensor_copy(du[:, :g, :], u_ps[:, :g, :])
                                nc.sync.dma_start(
                                    out=dbg_u[:, :g * F_out],
                                    in_=du[:, :g, :].rearrange("p g f -> p (g f)"))
                                de = ck.tile([128, G, H + F_out], dt.float32, tag="de")
                                nc.vector.tensor_copy(de[:, :g, :], eav[:, :g, :])
                                nc.sync.dma_start(
                                    out=dbg_eav[:, :g * (H + F_out)],
                                    in_=de[:, :g, :].rearrange("p g f -> p (g f)"))
                                da = ck.tile([128, G, F_out], dt.float32, tag="da")
                                nc.vector.tensor_copy(da[:, :g, :], amul[:, :g, :])
                                nc.sync.dma_start(
                                    out=dbg_amul[:, :g * F_out],
                                    in_=da[:, :g, :].rearrange("p g f -> p (g f)"))
                            _push(("so", [
                                (so_ps[:, jb, :], q_t[:, k0 + j, :], eav[:, j, :],
                                 k0 + j == 0, k0 + j == cbk - 1)
                                for j in range(g)]))
                    # drain pair accumulators to SBUF (deferred, after last so)
                    _push(("drain", (pair, so_ps)))

                _flush()
                # ---- end-pass for this segment: normalize + LN (+ELU)
                p0, p1 = int(seg_pair_off[seg]), int(seg_pair_off[seg + 1])
                P2 = 2 * (p1 - p0)
                b0 = 2 * p0
                s_v = sal[:, p0:p1, :, 0:H].rearrange("p a two h -> p (a two) h")
                num_v = sal[:, p0:p1, :, H:].rearrange(
                    "p a two (h c) -> p (a two) h c", h=H)
                inv = ep.tile([128, P2, H], dt.float32, tag="inv")
                nc.vector.tensor_scalar(inv[:], s_v, 1e-16, None, op0=Op.add)
                nc.vector.reciprocal(inv[:], inv[:])
                h_t = ep.tile([128, P2, F_out], dt.float32, tag="h")
                nc.vector.tensor_tensor(
                    h_t[:].rearrange("p B (h c) -> p B h c", h=H),
                    num_v,
                    inv[:].rearrange("p B (h o) -> p B h o", o=1)
                    .to_broadcast([128, P2, H, C]),
                    op=Op.mult)
                if split_P is not None:
                    nc.gpsimd.tensor_tensor(
                        h_t[:], h_t[:],
                        sinv_sb[:].rearrange("p (o f) -> p o f", o=1)
                        .to_broadcast([128, P2, F_out]), op=Op.mult)
                nc.vector.tensor_tensor(h_t[:], h_t[:], xr_sb[:, b0:b0 + P2, :],
                                        op=Op.add)
                mu = ep.tile([128, P2, 1], dt.float32, tag="mu")
                nc.vector.tensor_reduce(mu[:], h_t[:], axis=mybir.AxisListType.X,
                                        op=Op.add)
                nc.vector.tensor_scalar(mu[:], mu[:], 1.0 / F_out, None, op0=Op.mult)
                xc = ep.tile([128, P2, F_out], dt.float32, tag="xc")
                nc.vector.tensor_tensor(xc[:], h_t[:],
                                        mu[:].to_broadcast([128, P2, F_out]),
                                        op=Op.subtract)
                sq = ep.tile([128, P2, F_out], dt.float32, tag="sq")
                nc.gpsimd.tensor_tensor(sq[:], xc[:], xc[:], op=Op.mult)
                var = ep.tile([128, P2, 1], dt.float32, tag="var")
                nc.vector.tensor_reduce(var[:], sq[:], axis=mybir.AxisListType.X,
                                        op=Op.add)
                nc.vector.tensor_scalar(var[:], var[:], 1.0 / F_out, LN_EPS,
                                        op0=Op.mult, op1=Op.add)
                rstd = ep.tile([128, P2, 1], dt.float32, tag="rstd")
                nc.vector.reciprocal(rstd[:], var[:])
                nc.scalar.activation(rstd[:], rstd[:], A.Sqrt)
                nc.vector.tensor_tensor(xc[:], xc[:],
                                        rstd[:].to_broadcast([128, P2, F_out]),
                                        op=Op.mult)
                nc.gpsimd.tensor_tensor(
                    xc[:], xc[:],
                    g_sb[:].rearrange("p (o f) -> p o f", o=1)
                    .to_broadcast([128, P2, F_out]), op=Op.mult)
                nc.gpsimd.tensor_tensor(
                    xc[:], xc[:],
                    b_sb[:].rearrange("p (o f) -> p o f", o=1)
                    .to_broadcast([128, P2, F_out]), op=Op.add)
                if layer == 1:
                    m0 = ep.tile([128, P2, F_out], dt.float32, tag="sq")
                    nc.vector.tensor_scalar(m0[:], xc[:], 0.0, None, op0=Op.min)
                    ex = ep.tile([128, P2, F_out], dt.float32, tag="h")
                    nc.scalar.activation(ex[:], m0[:], A.Exp)
                    nc.vector.scalar_tensor_tensor(xc[:], ex[:], -1.0, xc[:],
                                                   op0=Op.add, op1=Op.max)
                nc.sync.dma_start(
                    out=hout[b0 * BLK:(b0 + P2) * BLK, :]
                    .rearrange("(B p) f -> p B f", p=128),
                    in_=xc[:])
            if debug:
                nc.sync.dma_start(
                    out=dbg_xr[:],
                    in_=xr_sb[:].rearrange("p B f -> p (B f)"))
                nc.sync.dma_start(
                    out=dbg_sal[:],
                    in_=sal[:].rearrange("p a two f -> p (a two f)"))
            ctx.__exit__(None, None, None)
    nc.compile()
    return nc


def _fold_att(att, F_out):
    """Column permutation + scales folding single-head attention into the
    weights: pos-att columns first (alpha 0.2), neg-att columns (alpha 5,
    with the extra 0.2 folded into the scale).  Returns (perm, s, P)."""
    att = np.asarray(att, np.float32).reshape(-1)
    perm = np.argsort(att < 0, kind="stable")
    P = int((att >= 0).sum())
    s = np.where(att >= 0, att, NEG_SLOPE * att)[perm]
    return perm, s, P


def _make_in_maps(meta, x_full, W_l, W_r, att, bias, g_ln, b_ln, F_out,
                  dev_q=False, fold=None):
    def rep(v):
        return np.tile(np.asarray(v, np.float32).reshape(1, F_out), (128, 1))

    wl_b = np.asarray(W_l, np.float32)
    wr_b = np.asarray(W_r, np.float32)
    bias_v, g_v, b_v = bias, g_ln, b_ln
    extra = {}
    if fold is not None:
        perm, s, P = fold
        wrx = wr_b[:, perm].astype(ml_dtypes.bfloat16)
        wl_b = wl_b[:, perm] * s.reshape(1, -1)
        wr_b = wr_b[:, perm] * s.reshape(1, -1)
        bias_v = np.asarray(bias, np.float32)[perm]
        g_v = np.asarray(g_ln, np.float32)[perm]
        b_v = np.asarray(b_ln, np.float32)[perm]
        extra = {"wrx": wrx, "sinv": rep(1.0 / s)}
    else:
        extra = {"att": rep(att).astype(ml_dtypes.bfloat16)}
    wl_b = wl_b.astype(ml_dtypes.bfloat16)
    wr_b = wr_b.astype(ml_dtypes.bfloat16)
    xb = np.asarray(x_full, np.float32).astype(ml_dtypes.bfloat16)
    maps = []
    for c in range(N_CORES):
        core = meta["cores"][c]
        xts, xtd = _perm_streams(meta, xb, core)
        m = {
            "xts": xts, "xtd": xtd,
            "xTs": _slice_stream(x_full, c),
            "wl": wl_b, "wr": wr_b, "bias": rep(bias_v),
            "g": rep(g_v), "b": rep(b_v), "mask": core["mask"],
            **extra,
        }
        if dev_q:
            m["dcol"] = core["dcol"]
        else:
            m["q"] = core["q"]
        maps.append(m)
    return maps


def _maybe_install_ntff_hook():
    try:
        import types
        import antenv
        if "antenv.axon_hooks" in sys.modules:
            return True
        mod = types.ModuleType("antenv.axon_hooks")
        state = {"hook": None}
        mod.set_axon_ntff_profile_hook = lambda h: state.__setitem__("hook", h)
        mod.get_axon_ntff_profile_hook = lambda: state["hook"]
        sys.modules["antenv.axon_hooks"] = mod
        antenv.axon_hooks = mod
        from trn_agent_boot.trn_boot import _ntff_profile_via_ctypes
        mod.set_axon_ntff_profile_hook(
            _ntff_profile_via_ctypes("/opt/axon/libaxon_pjrt.so"))
        return True
    except Exception:
        return False


def _run_with_retry(nc, maps, core_ids, trace, tries=3):
    last = None
    for i in range(tries):
        try:
            return run_bass_kernel_spmd(nc, maps, core_ids, trace=trace)
        except Exception as e:  # device flake: retry (fresh exec usually recovers)
            last = e
            if i == tries - 1:
                raise
    raise last


def kernel(**inputs):
    global LAST_EXEC_NS
    LAST_EXEC_NS = {}
    trace = os.environ.get("GAT_TRACE", "0") == "1"
    if trace:
        trace = _maybe_install_ntff_hook()

    x = np.asarray(inputs["x"], np.float32)
    edge_index = np.asarray(inputs["edge_index"])
    meta = _host_prep(edge_index)
    core_ids = list(range(N_CORES))
    debug = os.environ.get("GAT_DEBUG", "0") == "1"

    # ---- layer 1
    nc1 = _build_layer(meta, F_OUT1, H1, layer=1, debug=debug)
    maps1 = _make_in_maps(meta, x, inputs["Wl1"], inputs["Wr1"],
                          np.asarray(inputs["att1"], np.float32).reshape(-1),
                          inputs["bias1"], inputs["g1"], inputs["b1"], F_OUT1)
    res1 = _run_with_retry(nc1, maps1, core_ids, trace)
    h1 = np.concatenate([res1.results[c]["hout"][:S] for c in range(N_CORES)],
                        axis=0)
    if trace:
        LAST_EXEC_NS["layer1"] = res1.exec_time_ns
    if debug:
        np.savez("/root/problem/work/dbg_l1.npz",
                 **{k: np.asarray(res1.results[0][k]) for k in
                    ("dbg_u", "dbg_eav", "dbg_xr", "dbg_sal", "dbg_amul", "hout")})
    if os.environ.get("GAT_L1_ONLY", "0") == "1":
        np.save("/root/problem/work/h1_hw.npy", h1)
        return np.zeros((N, F_OUT2), np.float32)

    # ---- layer 2 (attention folded into the weights, single head)
    att2 = np.asarray(inputs["att2"], np.float32).reshape(-1)
    fold = _fold_att(att2, F_OUT2)
    nc2 = _build_layer(meta, F_OUT2, 1, layer=2, debug=debug, split_P=fold[2])
    maps2 = _make_in_maps(meta, h1, inputs["Wl2"], inputs["Wr2"], att2,
                          inputs["bias2"], inputs["g2"], inputs["b2"], F_OUT2,
                          dev_q=True, fold=fold)
    res2 = _run_with_retry(nc2, maps2, core_ids, trace)
    out_p = np.concatenate([res2.results[c]["hout"][:S] for c in range(N_CORES)],
                           axis=0)
    out = np.empty_like(out_p)
    out[:, fold[0]] = out_p  # undo the column permutation
    if trace:
        LAST_EXEC_NS["layer2"] = res2.exec_time_ns
    if debug:
        np.savez("/root/problem/work/dbg_l2.npz",
                 h1=h1,
                 **{k: np.asarray(res2.results[0][k]) for k in
                    ("dbg_u", "dbg_eav", "dbg_xr", "dbg_sal", "dbg_amul", "hout")})
    return out.astype(np.float32)


# revision 46
# speedup vs baseline: 1.1806x; 1.1806x over previous
"""GATv2 2-layer GNN on 8 Trainium2 NeuronCores (self-contained).

Sharding: destination nodes (and their incident edges) are partitioned
across the 8 cores; weights replicated.  The host pre-permutes node
features into per-edge streaming order (halo exchange + gather done on
the host), so the device never does an indexed gather:

  - per edge-chunk of 128: u = x[src].T @ Wl + x[dst].T @ Wr accumulated
    in PSUM (two streaming matmuls; the per-edge operands arrive as
    plain sequential DMA).
  - logits: Prelu(u) on ScalarE, * att + per-head reduce on
    GpSimd/VectorE, Exp on ScalarE.
  - weighted sums: one-hot scatter matmul so += Q @ [ea | ea*u] where Q
    is a host-built 0/1 matrix (dst-in-block per edge).  Both the
    numerator sum_e ea*u and denominator sum_e ea accumulate in PSUM.
  - out[d] = (sum_e ea*u)/(sum_e ea) - xr[d]  (softmax weights sum to 1,
    so the xr[dst] part of u contributes exactly xr[d]; subtract it).
    xr = x_slice @ Wr is masked to 0 for edge-less nodes.
  - layernorm (+ELU for layer 1) runs in 4 batched end-passes over
    ~12-block segments, avoiding per-block scalar-engine table thrash
    (only Prelu/Exp/Copy/Sqrt are used).

The h1 exchange between the two layers is done on the host.
"""
import os
import sys
import numpy as np

sys.path.insert(0, "/opt/trn_rl_repo")

import ml_dtypes
import concourse.bacc as bacc
import concourse.mybir as mybir
from concourse.tile import TileContext
from concourse.bass_utils import run_bass_kernel_spmd

dt = mybir.dt
A = mybir.ActivationFunctionType
Op = mybir.AluOpType

N, E = 50000, 800000
F_IN, F_H, H1, F_OUT2 = 128, 16, 8, 64
F_OUT1 = H1 * F_H  # 128
NEG_SLOPE = 0.2
LN_EPS = 1e-5
N_CORES = 8
BLK = 128
S = N // N_CORES          # 6250 dst nodes per core
NB = 50                   # 49 live blocks + 1 pad block
NPAIR = NB // 2
SEG_PAIRS = (7, 6, 6, 6)  # end-pass segments (pairs)
G = 4                     # chunks per inner group
LAG = 4                   # groups of scatter-matmul deferral (sw pipeline)

# exec-time info from the most recent kernel() call (for test harnesses)
LAST_EXEC_NS = {}


# ---------------------------------------------------------------- host prep
def _host_prep(edge_index):
    """Edge layout shared by both layers: per core, edges sorted by dst,
    grouped into 128-dst blocks, chunked by 128 edges.  Returns per-core
    column->node permutations (src/dst), the scatter one-hot q, and the
    has-edge mask."""
    src = np.asarray(edge_index[0], dtype=np.int64)
    dst = np.asarray(edge_index[1], dtype=np.int64)

    order = np.argsort(dst, kind="stable")
    src_s, dst_s = src[order], dst[order]
    core_of = dst_s // S

    per_core = []
    counts = np.zeros((N_CORES, NB), dtype=np.int64)
    for c in range(N_CORES):
        m = core_of == c
        sc, dc = src_s[m], dst_s[m] - c * S
        b_of = dc // BLK
        counts[c] = np.bincount(b_of, minlength=NB)
        per_core.append((sc, dc, b_of))

    cblk = np.maximum(1, (counts.max(axis=0) + BLK - 1) // BLK)  # [NB]
    offC = np.concatenate([[0], np.cumsum(cblk)])
    C_total = int(offC[-1])

    cores = []
    for c in range(N_CORES):
        sc, dc, b_of = per_core[c]
        # edges are dst-sorted, so per-block runs are contiguous
        block_start = np.concatenate([[0], np.cumsum(counts[c])])
        j_in_block = np.arange(len(sc)) - block_start[b_of]
        col = (offC[b_of] + j_in_block // BLK) * BLK + j_in_block % BLK

        src_ids = np.zeros(C_total * BLK, dtype=np.int64)
        dst_ids = np.zeros(C_total * BLK, dtype=np.int64)
        src_ids[col] = sc
        dst_ids[col] = sc * 0 + (dc + c * S)
        q = np.zeros((BLK, C_total * BLK), dtype=ml_dtypes.bfloat16)
        lane = col % BLK
        chunk = col // BLK
        q.reshape(-1)[lane * (C_total * BLK) + chunk * BLK + (dc % BLK)] = 1.0
        dcol = np.full((BLK, C_total), -1.0, dtype=ml_dtypes.bfloat16)
        dcol[lane, chunk] = (dc % BLK).astype(np.float32)

        deg = np.bincount(dc, minlength=NB * BLK)[: NB * BLK]
        live = (np.arange(NB * BLK) < S) & (deg > 0)
        mask = np.ascontiguousarray(
            live.reshape(NB, BLK).T.astype(np.float32))  # [128, NB]
        cores.append(dict(src_ids=src_ids, dst_ids=dst_ids, q=q, dcol=dcol,
                          mask=mask))

    return dict(cblk=cblk, offC=offC, C_total=C_total, cores=cores)


def _perm_streams(meta, x_full, core):
    """Per-edge feature streams for one core: x[src].T and x[dst].T as
    [128, C_total*128] bf16."""
    xb = x_full if x_full.dtype == ml_dtypes.bfloat16 else \
        np.asarray(x_full, np.float32).astype(ml_dtypes.bfloat16)
    xts = np.ascontiguousarray(xb[core["src_ids"]].T)
    xtd = np.ascontiguousarray(xb[core["dst_ids"]].T)
    return xts, xtd


def _slice_stream(x_full, c):
    """Own dst-slice, transposed+padded to [128, NB*128] bf16 (for xr)."""
    sl = np.zeros((NB * BLK, x_full.shape[1]), dtype=np.float32)
    sl[:S] = np.asarray(x_full[c * S:(c + 1) * S], np.float32)
    return np.ascontiguousarray(sl.T).astype(ml_dtypes.bfloat16)


# ---------------------------------------------------------------- builder
def _build_layer(meta, F_out, H, layer, debug=False, split_P=None):
    """split_P: if not None, attention weights are host-folded into Wl/Wr
    (single-head only); columns [0:split_P] use Prelu alpha=0.2, the rest
    alpha=5.0, and the logits are a plain per-head sum of the Prelu output.
    The end-pass multiplies by the host-provided 1/s compensation."""
    cblk, offC, C_total = meta["cblk"], meta["offC"], meta["C_total"]
    C = F_out // H

    nc = bacc.Bacc("TRN2", target_bir_lowering=False, debug=False,
                   num_devices=N_CORES)
    xts_d = nc.dram_tensor("xts", [128, C_total * BLK], dt.bfloat16, kind="ExternalInput").ap()
    xtd_d = nc.dram_tensor("xtd", [128, C_total * BLK], dt.bfloat16, kind="ExternalInput").ap()
    dev_q = layer == 2  # build scatter one-hot on device (layer2 is DMA-bound)
    if dev_q:
        dcol_d = nc.dram_tensor("dcol", [128, C_total], dt.bfloat16, kind="ExternalInput").ap()
    else:
        q_d = nc.dram_tensor("q", [128, C_total * BLK], dt.bfloat16, kind="ExternalInput").ap()
    xTs = nc.dram_tensor("xTs", [128, NB * BLK], dt.bfloat16, kind="ExternalInput").ap()
    wl = nc.dram_tensor("wl", [128, F_out], dt.bfloat16, kind="ExternalInput").ap()
    wr = nc.dram_tensor("wr", [128, F_out], dt.bfloat16, kind="ExternalInput").ap()
    if split_P is not None:
        wrx_in = nc.dram_tensor("wrx", [128, F_out], dt.bfloat16, kind="ExternalInput").ap()
        sinv_in = nc.dram_tensor("sinv", [128, F_out], dt.float32, kind="ExternalInput").ap()
    else:
        att_in = nc.dram_tensor("att", [128, F_out], dt.bfloat16, kind="ExternalInput").ap()
    bias_in = nc.dram_tensor("bias", [128, F_out], dt.float32, kind="ExternalInput").ap()
    g_in = nc.dram_tensor("g", [128, F_out], dt.float32, kind="ExternalInput").ap()
    b_in = nc.dram_tensor("b", [128, F_out], dt.float32, kind="ExternalInput").ap()
    mask_in = nc.dram_tensor("mask", [128, NB], dt.float32, kind="ExternalInput").ap()
    hout = nc.dram_tensor("hout", [NB * BLK, F_out], dt.float32, kind="ExternalOutput").ap()
    if debug:
        dbg_u = nc.dram_tensor("dbg_u", [128, G * F_out], dt.float32, kind="ExternalOutput").ap()
        dbg_eav = nc.dram_tensor("dbg_eav", [128, G * (H + F_out)], dt.float32, kind="ExternalOutput").ap()
        dbg_xr = nc.dram_tensor("dbg_xr", [128, NB * F_out], dt.float32, kind="ExternalOutput").ap()
        dbg_sal = nc.dram_tensor("dbg_sal", [128, NPAIR * 2 * (H + F_out)], dt.float32, kind="ExternalOutput").ap()
        dbg_amul = nc.dram_tensor("dbg_amul", [128, G * F_out], dt.float32, kind="ExternalOutput").ap()

    with TileContext(nc) as tc:
        with (
            tc.tile_pool(name="con", bufs=1) as con,
            tc.tile_pool(name="st", bufs=3) as st,
            tc.tile_pool(name="ck", bufs=6) as ck,
            tc.tile_pool(name="ep", bufs=2) as ep,
            tc.tile_pool(name="ps_u", bufs=5, space="PSUM") as ps_u,
            tc.tile_pool(name="ps_acc", bufs=2, space="PSUM") as ps_acc,
        ):
            # constants
            wl_sb = con.tile([128, F_out], dt.bfloat16)
            nc.sync.dma_start(out=wl_sb[:], in_=wl[:])
            wr_sb = con.tile([128, F_out], dt.bfloat16)
            nc.sync.dma_start(out=wr_sb[:], in_=wr[:])
            if split_P is not None:
                wrx_sb = con.tile([128, F_out], dt.bfloat16)
                nc.sync.dma_start(out=wrx_sb[:], in_=wrx_in[:])
                sinv_sb = con.tile([128, F_out], dt.float32)
                nc.sync.dma_start(out=sinv_sb[:], in_=sinv_in[:])
            else:
                wrx_sb = wr_sb
                att_sb = con.tile([128, F_out], dt.bfloat16)
                nc.sync.dma_start(out=att_sb[:], in_=att_in[:])
            bias_sb = con.tile([128, F_out], dt.float32)
            nc.sync.dma_start(out=bias_sb[:], in_=bias_in[:])
            g_sb = con.tile([128, F_out], dt.float32)
            nc.sync.dma_start(out=g_sb[:], in_=g_in[:])
            b_sb = con.tile([128, F_out], dt.float32)
            nc.sync.dma_start(out=b_sb[:], in_=b_in[:])
            mask_sb = con.tile([128, NB], dt.float32)
            nc.sync.dma_start(out=mask_sb[:], in_=mask_in[:])
            sal = con.tile([128, NPAIR, 2, H + F_out], dt.float32)
            xr_sb = con.tile([128, NB, F_out], dt.float32)
            if dev_q:
                dcol_sb = con.tile([128, C_total], dt.bfloat16)
                nc.sync.dma_start(out=dcol_sb[:], in_=dcol_d[:])
                iota_row = con.tile([128, 128], dt.int32)
                nc.gpsimd.iota(iota_row[:], pattern=[[1, 128]], base=0,
                               channel_multiplier=0)
                iota_row_b = con.tile([128, 128], dt.bfloat16)
                nc.vector.tensor_copy(iota_row_b[:], iota_row[:])
                iota_bc1 = iota_row_b[:].rearrange("p (o f) -> p o f", o=1)

            # xr = x_slice @ Wr, masked to 0 for edge-less dst rows; then
            # xr_sb <- bias - xr so the end-pass needs one add, not two ops.
            ctx = nc.named_scope("xr"); ctx.__enter__()
            XB = G  # reuse the edge-phase PSUM tag/shape
            for t0 in range(0, NB, XB):
                n = min(XB, NB - t0)
                xs_t = st.tile([128, XB, 128], dt.bfloat16, tag="xs")
                nc.sync.dma_start(out=xs_t[:, :n, :],
                                  in_=xTs[:, t0 * 128:(t0 + n) * 128])
                pd = ps_u.tile([128, G, F_out], dt.float32, tag="ups")
                for i in range(n):
                    nc.tensor.matmul(pd[:, i, :], xs_t[:, i, :], wrx_sb[:],
                                     start=True, stop=True)
                    nc.scalar.activation(xr_sb[:, t0 + i, :], pd[:, i, :], A.Copy,
                                         scale=mask_sb[:, t0 + i:t0 + i + 1])
            nc.vector.scalar_tensor_tensor(
                xr_sb[:], xr_sb[:], -1.0,
                bias_sb[:].rearrange("p (o f) -> p o f", o=1)
                .to_broadcast([128, NB, F_out]),
                op0=Op.mult, op1=Op.add)
            ctx.__exit__(None, None, None)

            ctx = nc.named_scope("edge"); ctx.__enter__()
            if split_P is None:
                att_bc1 = att_sb[:].rearrange("p (o f) -> p o f", o=1)
            seg_pair_off = np.concatenate([[0], np.cumsum(SEG_PAIRS)])

            # deferred emission of scatter matmuls + pair drains: keeps the
            # in-order PE queue LAG groups ahead of the eav dependency
            fifo = []

            def _emit(item):
                if item[0] == "so":
                    for ps_ap, q_ap, eav_ap, st_, sp_ in item[1]:
                        nc.tensor.matmul(ps_ap, q_ap, eav_ap, start=st_, stop=sp_)
                else:
                    pair_, so_tile = item[1]
                    nc.scalar.activation(sal[:, pair_, :, :], so_tile[:], A.Copy)

            def _push(item):
                fifo.append(item)
                n_so = sum(1 for it in fifo if it[0] == "so")
                while n_so > LAG:
                    it = fifo.pop(0)
                    _emit(it)
                    if it[0] == "so":
                        n_so -= 1

            def _flush():
                while fifo:
                    _emit(fifo.pop(0))

            for seg in range(len(SEG_PAIRS)):
                for pair in range(seg_pair_off[seg], seg_pair_off[seg + 1]):
                    so_ps = ps_acc.tile([128, 2, H + F_out], dt.float32, tag="sops")
                    for jb in range(2):
                        b = 2 * pair + jb
                        cbk = int(cblk[b])
                        c0 = int(offC[b]) * BLK
                        xts_t = st.tile([128, cbk, 128], dt.bfloat16, tag="xts")
                        nc.sync.dma_start(out=xts_t[:], in_=xts_d[:, c0:c0 + cbk * BLK])
                        xtd_t = st.tile([128, cbk, 128], dt.bfloat16, tag="xtd")
                        nc.sync.dma_start(out=xtd_t[:], in_=xtd_d[:, c0:c0 + cbk * BLK])
                        if dev_q:
                            q_t = st.tile([128, cbk, 128], dt.bfloat16, tag="qt")
                            cc0 = int(offC[b])
                            nc.vector.tensor_tensor(
                                q_t[:],
                                iota_bc1.to_broadcast([128, cbk, 128]),
                                dcol_sb[:, cc0:cc0 + cbk]
                                .rearrange("p (k o) -> p k o", o=1)
                                .to_broadcast([128, cbk, 128]),
                                op=Op.is_equal)
                        else:
                            q_t = st.tile([128, cbk, 128], dt.bfloat16, tag="qt")
                            nc.sync.dma_start(out=q_t[:], in_=q_d[:, c0:c0 + cbk * BLK])

                        for k0 in range(0, cbk, G):
                            g = min(G, cbk - k0)
                            u_ps = ps_u.tile([128, G, F_out], dt.float32, tag="ups")
                            for j in range(g):
                                k = k0 + j
                                nc.tensor.matmul(u_ps[:, j, :], xts_t[:, k, :],
                                                 wl_sb[:], start=True, stop=False)
                                nc.tensor.matmul(u_ps[:, j, :], xtd_t[:, k, :],
                                                 wr_sb[:], start=False, stop=True)
                            lr = ck.tile([128, G, F_out], dt.bfloat16, tag="lr")
                            if split_P is not None:
                                if split_P > 0:
                                    nc.scalar.activation(
                                        lr[:, :g, 0:split_P],
                                        u_ps[:, :g, 0:split_P],
                                        A.Prelu, alpha=NEG_SLOPE)
                                if split_P < F_out:
                                    nc.scalar.activation(
                                        lr[:, :g, split_P:],
                                        u_ps[:, :g, split_P:],
                                        A.Prelu, alpha=1.0 / NEG_SLOPE)
                                amul = lr
                            else:
                                nc.scalar.activation(lr[:, :g, :], u_ps[:, :g, :],
                                                     A.Prelu, alpha=NEG_SLOPE)
                                amul = ck.tile([128, G, F_out], dt.bfloat16, tag="amul")
                                nc.gpsimd.tensor_tensor(
                                    amul[:, :g, :], lr[:, :g, :],
                                    att_bc1.to_broadcast([128, g, F_out]), op=Op.mult)
                            a4 = ck.tile([128, G, H], dt.float32, tag="a4")
                            nc.vector.tensor_reduce(
                                a4[:, :g, :],
                                amul[:, :g, :].rearrange("p g (h c) -> p g h c", h=H),
                                axis=mybir.AxisListType.X, op=Op.add)
                            eav = ck.tile([128, G, H + F_out], dt.bfloat16, tag="eav")
                            nc.scalar.activation(eav[:, :g, 0:H], a4[:, :g, :], A.Exp)
                            nc.vector.tensor_tensor(
                                eav[:, :g, H:].rearrange("p g (h c) -> p g h c", h=H),
                                u_ps[:, :g, :].rearrange("p g (h c) -> p g h c", h=H),
                                eav[:, :g, 0:H].rearrange("p g (h o) -> p g h o", o=1)
                                .to_broadcast([128, g, H, C]),
                                op=Op.mult)
                            if debug and b == 0 and k0 == 0:
                                _flush()
                                du = ck.tile([128, G, F_out], dt.float32, tag="du")
                                nc.vector.tensor_copy(du[:, :g, :], u_ps[:, :g, :])
                                nc.sync.dma_start(
                                    out=dbg_u[:, :g * F_out],
                                    in_=du[:, :g, :].rearrange("p g f -> p (g f)"))
                                de = ck.tile([128, G, H + F_out], dt.float32, tag="de")
                                nc.vector.tensor_copy(de[:, :g, :], eav[:, :g, :])
                                nc.sync.dma_start(
                                    out=dbg_eav[:, :g * (H + F_out)],
                                    in_=de[:, :g, :].rearrange("p g f -> p (g f)"))
                                da = ck.tile([128, G, F_out], dt.float32, tag="da")
                                nc.vector.tensor_copy(da[:, :g, :], amul[:, :g, :])
                                nc.sync.dma_start(
                                    out=dbg_amul[:, :g * F_out],
                                    in_=da[:, :g, :].rearrange("p g f -> p (g f)"))
                            _push(("so", [
                                (so_ps[:, jb, :], q_t[:, k0 + j, :], eav[:, j, :],
                                 k0 + j == 0, k0 + j == cbk - 1)
                                for j in range(g)]))
                    # drain pair accumulators to SBUF (deferred, after last so)
                    _push(("drain", (pair, so_ps)))

                _flush()
                # ---- end-pass for this segment: normalize + LN (+ELU)
                p0, p1 = int(seg_pair_off[seg]), int(seg_pair_off[seg + 1])
                P2 = 2 * (p1 - p0)
                b0 = 2 * p0
                s_v = sal[:, p0:p1, :, 0:H].rearrange("p a two h -> p (a two) h")
                num_v = sal[:, p0:p1, :, H:].rearrange(
                    "p a two (h c) -> p (a two) h c", h=H)
                inv = ep.tile([128, P2, H], dt.float32, tag="inv")
                nc.vector.tensor_scalar(inv[:], s_v, 1e-16, None, op0=Op.add)
                nc.vector.reciprocal(inv[:], inv[:])
                h_t = ep.tile([128, P2, F_out], dt.float32, tag="h")
                nc.vector.tensor_tensor(
                    h_t[:].rearrange("p B (h c) -> p B h c", h=H),
                    num_v,
                    inv[:].rearrange("p B (h o) -> p B h o", o=1)
                    .to_broadcast([128, P2, H, C]),
                    op=Op.mult)
                if split_P is not None:
                    nc.gpsimd.tensor_tensor(
                        h_t[:], h_t[:],
                        sinv_sb[:].rearrange("p (o f) -> p o f", o=1)
                        .to_broadcast([128, P2, F_out]), op=Op.mult)
                nc.vector.tensor_tensor(h_t[:], h_t[:], xr_sb[:, b0:b0 + P2, :],
                                        op=Op.add)
                mu = ep.tile([128, P2, 1], dt.float32, tag="mu")
                nc.vector.tensor_reduce(mu[:], h_t[:], axis=mybir.AxisListType.X,
                                        op=Op.add)
                nc.vector.tensor_scalar(mu[:], mu[:], 1.0 / F_out, None, op0=Op.mult)
                xc = ep.tile([128, P2, F_out], dt.float32, tag="xc")
                nc.vector.tensor_tensor(xc[:], h_t[:],
                                        mu[:].to_broadcast([128, P2, F_out]),
                                        op=Op.subtract)
                sq = ep.tile([128, P2, F_out], dt.float32, tag="sq")
                nc.gpsimd.tensor_tensor(sq[:], xc[:], xc[:], op=Op.mult)
                var = ep.tile([128, P2, 1], dt.float32, tag="var")
                nc.vector.tensor_reduce(var[:], sq[:], axis=mybir.AxisListType.X,
                                        op=Op.add)
                nc.vector.tensor_scalar(var[:], var[:], 1.0 / F_out, LN_EPS,
                                        op0=Op.mult, op1=Op.add)
                rstd = ep.tile([128, P2, 1], dt.float32, tag="rstd")
                nc.vector.reciprocal(rstd[:], var[:])
                nc.scalar.activation(rstd[:], rstd[:], A.Sqrt)
                nc.vector.tensor_tensor(xc[:], xc[:],
                                        rstd[:].to_broadcast([128, P2, F_out]),
                                        op=Op.mult)
                nc.gpsimd.tensor_tensor(
                    xc[:], xc[:],
                    g_sb[:].rearrange("p (o f) -> p o f", o=1)
                    .to_broadcast([128, P2, F_out]), op=Op.mult)
                nc.gpsimd.tensor_tensor(
                    xc[:], xc[:],
                    b_sb[:].rearrange("p (o f) -> p o f", o=1)
                    .to_broadcast([128, P2, F_out]), op=Op.add)
                if layer == 1:
                    m0 = ep.tile([128, P2, F_out], dt.float32, tag="sq")
                    nc.vector.tensor_scalar(m0[:], xc[:], 0.0, None, op0=Op.min)
                    ex = ep.tile([128, P2, F_out], dt.float32, tag="h")
                    nc.scalar.activation(ex[:], m0[:], A.Exp)
                    nc.vector.scalar_tensor_tensor(xc[:], ex[:], -1.0, xc[:],
                                                   op0=Op.add, op1=Op.max)
                nc.sync.dma_start(
                    out=hout[b0 * BLK:(b0 + P2) * BLK, :]
                    .rearrange("(B p) f -> p B f", p=128),
                    in_=xc[:])
            if debug:
                nc.sync.dma_start(
                    out=dbg_xr[:],
                    in_=xr_sb[:].rearrange("p B f -> p (B f)"))
                nc.sync.dma_start(
                    out=dbg_sal[:],
                    in_=sal[:].rearrange("p a two f -> p (a two f)"))
            ctx.__exit__(None, None, None)
    nc.compile()
    return nc


def _fold_att(att, F_out):
    """Column permutation + scales folding single-head attention into the
    weights: pos-att columns first (alpha 0.2), neg-att columns (alpha 5,
    with the extra 0.2 folded into the scale).  Returns (perm, s, P)."""
    att = np.asarray(att, np.float32).reshape(-1)
    perm = np.argsort(att < 0, kind="stable")
    P = int((att >= 0).sum())
    s = np.where(att >= 0, att, NEG_SLOPE * att)[perm]
    s = np.where(s == 0.0, 1e-20, s)  # guard 0*inf for exactly-zero att
    return perm, s, P


def _make_in_maps(meta, x_full, W_l, W_r, att, bias, g_ln, b_ln, F_out,
                  dev_q=False, fold=None):
    def rep(v):
        return np.tile(np.asarray(v, np.float32).reshape(1, F_out), (128, 1))

    wl_b = np.asarray(W_l, np.float32)
    wr_b = np.asarray(W_r, np.float32)
    bias_v, g_v, b_v = bias, g_ln, b_ln
    extra = {}
    if fold is not None:
        perm, s, P = fold
        wrx = wr_b[:, perm].astype(ml_dtypes.bfloat16)
        wl_b = wl_b[:, perm] * s.reshape(1, -1)
        wr_b = wr_b[:, perm] * s.reshape(1, -1)
        bias_v = np.asarray(bias, np.float32)[perm]
        g_v = np.asarray(g_ln, np.float32)[perm]
        b_v = np.asarray(b_ln, np.float32)[perm]
        extra = {"wrx": wrx, "sinv": rep(1.0 / s)}
    else:
        extra = {"att": rep(att).astype(ml_dtypes.bfloat16)}
    wl_b = wl_b.astype(ml_dtypes.bfloat16)
    wr_b = wr_b.astype(ml_dtypes.bfloat16)
    xb = np.asarray(x_full, np.float32).astype(ml_dtypes.bfloat16)
    maps = []
    for c in range(N_CORES):
        core = meta["cores"][c]
        xts, xtd = _perm_streams(meta, xb, core)
        m = {
            "xts": xts, "xtd": xtd,
            "xTs": _slice_stream(x_full, c),
            "wl": wl_b, "wr": wr_b, "bias": rep(bias_v),
            "g": rep(g_v), "b": rep(b_v), "mask": core["mask"],
            **extra,
        }
        if dev_q:
            m["dcol"] = core["dcol"]
        else:
            m["q"] = core["q"]
        maps.append(m)
    return maps


def _maybe_install_ntff_hook():
    try:
        import types
        import antenv
        if "antenv.axon_hooks" in sys.modules:
            return True
        mod = types.ModuleType("antenv.axon_hooks")
        state = {"hook": None}
        mod.set_axon_ntff_profile_hook = lambda h: state.__setitem__("hook", h)
        mod.get_axon_ntff_profile_hook = lambda: state["hook"]
        sys.modules["antenv.axon_hooks"] = mod
        antenv.axon_hooks = mod
        from trn_agent_boot.trn_boot import _ntff_profile_via_ctypes
        mod.set_axon_ntff_profile_hook(
            _ntff_profile_via_ctypes("/opt/axon/libaxon_pjrt.so"))
        return True
    except Exception:
        return False


def _run_with_retry(nc, maps, core_ids, trace, tries=3):
    last = None
    for i in range(tries):
        try:
            return run_bass_kernel_spmd(nc, maps, core_ids, trace=trace)
        except Exception as e:  # device flake: retry (fresh exec usually recovers)
            last = e
            if i == tries - 1:
                raise
    raise last


def kernel(**inputs):
    global LAST_EXEC_NS
    LAST_EXEC_NS = {}
    trace = os.environ.get("GAT_TRACE", "0") == "1"
    if trace:
        trace = _maybe_install_ntff_hook()

    x = np.asarray(inputs["x"], np.float32)
    edge_index = np.asarray(inputs["edge_index"])
    meta = _host_prep(edge_index)
    core_ids = list(range(N_CORES))
    debug = os.environ.get("GAT_DEBUG", "0") == "1"

    # ---- layer 1
    nc1 = _build_layer(meta, F_OUT1, H1, layer=1, debug=debug)
    maps1 = _make_in_maps(meta, x, inputs["Wl1"], inputs["Wr1"],
                          np.asarray(inputs["att1"], np.float32).reshape(-1),
                          inputs["bias1"], inputs["g1"], inputs["b1"], F_OUT1)
    res1 = _run_with_retry(nc1, maps1, core_ids, trace)
    h1 = np.concatenate([res1.results[c]["hout"][:S] for c in range(N_CORES)],
                        axis=0)
    if trace:
        LAST_EXEC_NS["layer1"] = res1.exec_time_ns
    if debug:
        np.savez("/root/problem/work/dbg_l1.npz",
                 **{k: np.asarray(res1.results[0][k]) for k in
                    ("dbg_u", "dbg_eav", "dbg_xr", "dbg_sal", "dbg_amul", "hout")})
    if os.environ.get("GAT_L1_ONLY", "0") == "1":
        np.save("/root/problem/work/h1_hw.npy", h1)
        return np.zeros((N, F_OUT2), np.float32)

    # ---- layer 2 (attention folded into the weights, single head)
    att2 = np.asarray(inputs["att2"], np.float32).reshape(-1)
    fold = _fold_att(att2, F_OUT2)
    nc2 = _build_layer(meta, F_OUT2, 1, layer=2, debug=debug, split_P=fold[2])
    maps2 = _make_in_maps(meta, h1, inputs["Wl2"], inputs["Wr2"], att2,
                          inputs["bias2"], inputs["g2"], inputs["b2"], F_OUT2,
                          dev_q=True, fold=fold)
    res2 = _run_with_retry(nc2, maps2, core_ids, trace)
    out_p = np.concatenate([res2.results[c]["hout"][:S] for c in range(N_CORES)],
                           axis=0)
    out = np.empty_like(out_p)
    out[:, fold[0]] = out_p  # undo the column permutation
    if trace:
        LAST_EXEC_NS["layer2"] = res2.exec_time_ns
    if debug:
        np.savez("/root/problem/work/dbg_l2.npz",
                 h1=h1,
                 **{k: np.asarray(res2.results[0][k]) for k in
                    ("dbg_u", "dbg_eav", "dbg_xr", "dbg_sal", "dbg_amul", "hout")})
    return out.astype(np.float32)


# revision 49
# speedup vs baseline: 1.2502x; 1.0590x over previous
"""GATv2 2-layer GNN on 8 Trainium2 NeuronCores (self-contained).

Sharding: destination nodes (and their incident edges) are partitioned
across the 8 cores; weights replicated.  The host pre-permutes node
features into per-edge streaming order (halo exchange + gather done on
the host), so the device never does an indexed gather:

  - per edge-chunk of 128: u = x[src].T @ Wl + x[dst].T @ Wr accumulated
    in PSUM (two streaming matmuls; the per-edge operands arrive as
    plain sequential DMA).
  - logits: Prelu(u) on ScalarE, * att + per-head reduce on
    GpSimd/VectorE, Exp on ScalarE.
  - weighted sums: one-hot scatter matmul so += Q @ [ea | ea*u] where Q
    is a host-built 0/1 matrix (dst-in-block per edge).  Both the
    numerator sum_e ea*u and denominator sum_e ea accumulate in PSUM.
  - out[d] = (sum_e ea*u)/(sum_e ea) - xr[d]  (softmax weights sum to 1,
    so the xr[dst] part of u contributes exactly xr[d]; subtract it).
    xr = x_slice @ Wr is masked to 0 for edge-less nodes.
  - layernorm (+ELU for layer 1) runs in 4 batched end-passes over
    ~12-block segments, avoiding per-block scalar-engine table thrash
    (only Prelu/Exp/Copy/Sqrt are used).

The h1 exchange between the two layers is done on the host.
"""
import os
import sys
import numpy as np

sys.path.insert(0, "/opt/trn_rl_repo")

import ml_dtypes
import concourse.bacc as bacc
import concourse.mybir as mybir
from concourse.tile import TileContext
from concourse.bass_utils import run_bass_kernel_spmd

dt = mybir.dt
A = mybir.ActivationFunctionType
Op = mybir.AluOpType

N, E = 50000, 800000
F_IN, F_H, H1, F_OUT2 = 128, 16, 8, 64
F_OUT1 = H1 * F_H  # 128
NEG_SLOPE = 0.2
LN_EPS = 1e-5
N_CORES = 8
BLK = 128
S = N // N_CORES          # 6250 dst nodes per core
NB = 50                   # 49 live blocks + 1 pad block
NPAIR = NB // 2
SEG_PAIRS = (7, 6, 6, 6)  # end-pass segments (pairs)
G = 4                     # chunks per inner group
LAG = 4                   # groups of scatter-matmul deferral (sw pipeline)

# exec-time info from the most recent kernel() call (for test harnesses)
LAST_EXEC_NS = {}


# ---------------------------------------------------------------- host prep
def _host_prep(edge_index):
    """Edge layout shared by both layers: per core, edges sorted by dst,
    grouped into 128-dst blocks, chunked by 128 edges.  Returns per-core
    column->node permutations (src/dst), the scatter one-hot q, and the
    has-edge mask."""
    src = np.asarray(edge_index[0], dtype=np.int64)
    dst = np.asarray(edge_index[1], dtype=np.int64)

    order = np.argsort(dst, kind="stable")
    src_s, dst_s = src[order], dst[order]
    core_of = dst_s // S

    per_core = []
    counts = np.zeros((N_CORES, NB), dtype=np.int64)
    for c in range(N_CORES):
        m = core_of == c
        sc, dc = src_s[m], dst_s[m] - c * S
        b_of = dc // BLK
        counts[c] = np.bincount(b_of, minlength=NB)
        per_core.append((sc, dc, b_of))

    cblk = np.maximum(1, (counts.max(axis=0) + BLK - 1) // BLK)  # [NB]
    offC = np.concatenate([[0], np.cumsum(cblk)])
    C_total = int(offC[-1])

    cores = []
    for c in range(N_CORES):
        sc, dc, b_of = per_core[c]
        # edges are dst-sorted, so per-block runs are contiguous
        block_start = np.concatenate([[0], np.cumsum(counts[c])])
        j_in_block = np.arange(len(sc)) - block_start[b_of]
        col = (offC[b_of] + j_in_block // BLK) * BLK + j_in_block % BLK

        src_ids = np.zeros(C_total * BLK, dtype=np.int64)
        dst_ids = np.zeros(C_total * BLK, dtype=np.int64)
        src_ids[col] = sc
        dst_ids[col] = sc * 0 + (dc + c * S)
        q = np.zeros((BLK, C_total * BLK), dtype=ml_dtypes.bfloat16)
        lane = col % BLK
        chunk = col // BLK
        q.reshape(-1)[lane * (C_total * BLK) + chunk * BLK + (dc % BLK)] = 1.0
        dcol = np.full((BLK, C_total), -1.0, dtype=ml_dtypes.bfloat16)
        dcol[lane, chunk] = (dc % BLK).astype(np.float32)

        deg = np.bincount(dc, minlength=NB * BLK)[: NB * BLK]
        live = (np.arange(NB * BLK) < S) & (deg > 0)
        mask = np.ascontiguousarray(
            live.reshape(NB, BLK).T.astype(np.float32))  # [128, NB]
        cores.append(dict(src_ids=src_ids, dst_ids=dst_ids, q=q, dcol=dcol,
                          mask=mask))

    return dict(cblk=cblk, offC=offC, C_total=C_total, cores=cores)


def _perm_streams(meta, x_full, core):
    """Per-edge feature streams for one core: x[src].T and x[dst].T as
    [128, C_total*128] bf16."""
    xb = x_full if x_full.dtype == ml_dtypes.bfloat16 else \
        np.asarray(x_full, np.float32).astype(ml_dtypes.bfloat16)
    xts = np.ascontiguousarray(xb[core["src_ids"]].T)
    xtd = np.ascontiguousarray(xb[core["dst_ids"]].T)
    return xts, xtd


def _slice_stream(x_full, c):
    """Own dst-slice, transposed+padded to [128, NB*128] bf16 (for xr)."""
    sl = np.zeros((NB * BLK, x_full.shape[1]), dtype=np.float32)
    sl[:S] = np.asarray(x_full[c * S:(c + 1) * S], np.float32)
    return np.ascontiguousarray(sl.T).astype(ml_dtypes.bfloat16)


# ---------------------------------------------------------------- builder
def _build_layer(meta, F_out, H, layer, debug=False, split_P=None):
    """split_P: if not None, attention weights are host-folded into Wl/Wr
    (single-head only); columns [0:split_P] use Prelu alpha=0.2, the rest
    alpha=5.0, and the logits are a plain per-head sum of the Prelu output.
    The end-pass multiplies by the host-provided 1/s compensation."""
    cblk, offC, C_total = meta["cblk"], meta["offC"], meta["C_total"]
    C = F_out // H

    nc = bacc.Bacc("TRN2", target_bir_lowering=False, debug=False,
                   num_devices=N_CORES)
    xts_d = nc.dram_tensor("xts", [128, C_total * BLK], dt.bfloat16, kind="ExternalInput").ap()
    xtd_d = nc.dram_tensor("xtd", [128, C_total * BLK], dt.bfloat16, kind="ExternalInput").ap()
    # layer2: hybrid scatter one-hot — even blocks streamed from host (DMA
    # has slack), odd blocks built on DVE (halves the q-build DVE cost)
    dev_q = layer == 2
    q_d = nc.dram_tensor("q", [128, C_total * BLK], dt.bfloat16, kind="ExternalInput").ap()
    if dev_q:
        dcol_d = nc.dram_tensor("dcol", [128, C_total], dt.bfloat16, kind="ExternalInput").ap()
    xTs = nc.dram_tensor("xTs", [128, NB * BLK], dt.bfloat16, kind="ExternalInput").ap()
    wl = nc.dram_tensor("wl", [128, F_out], dt.bfloat16, kind="ExternalInput").ap()
    wr = nc.dram_tensor("wr", [128, F_out], dt.bfloat16, kind="ExternalInput").ap()
    if split_P is not None:
        wrx_in = nc.dram_tensor("wrx", [128, F_out], dt.bfloat16, kind="ExternalInput").ap()
        sinv_in = nc.dram_tensor("sinv", [128, F_out], dt.float32, kind="ExternalInput").ap()
    else:
        att_in = nc.dram_tensor("att", [128, F_out], dt.bfloat16, kind="ExternalInput").ap()
    bias_in = nc.dram_tensor("bias", [128, F_out], dt.float32, kind="ExternalInput").ap()
    g_in = nc.dram_tensor("g", [128, F_out], dt.float32, kind="ExternalInput").ap()
    b_in = nc.dram_tensor("b", [128, F_out], dt.float32, kind="ExternalInput").ap()
    mask_in = nc.dram_tensor("mask", [128, NB], dt.float32, kind="ExternalInput").ap()
    hout = nc.dram_tensor("hout", [NB * BLK, F_out], dt.float32, kind="ExternalOutput").ap()
    if debug:
        dbg_u = nc.dram_tensor("dbg_u", [128, G * F_out], dt.float32, kind="ExternalOutput").ap()
        dbg_eav = nc.dram_tensor("dbg_eav", [128, G * (H + F_out)], dt.float32, kind="ExternalOutput").ap()
        dbg_xr = nc.dram_tensor("dbg_xr", [128, NB * F_out], dt.float32, kind="ExternalOutput").ap()
        dbg_sal = nc.dram_tensor("dbg_sal", [128, NPAIR * 2 * (H + F_out)], dt.float32, kind="ExternalOutput").ap()
        dbg_amul = nc.dram_tensor("dbg_amul", [128, G * F_out], dt.float32, kind="ExternalOutput").ap()

    with TileContext(nc) as tc:
        with (
            tc.tile_pool(name="con", bufs=1) as con,
            tc.tile_pool(name="st", bufs=3) as st,
            tc.tile_pool(name="ck", bufs=6) as ck,
            tc.tile_pool(name="ep", bufs=2) as ep,
            tc.tile_pool(name="ps_u", bufs=5, space="PSUM") as ps_u,
            tc.tile_pool(name="ps_acc", bufs=2, space="PSUM") as ps_acc,
        ):
            # constants
            wl_sb = con.tile([128, F_out], dt.bfloat16)
            nc.sync.dma_start(out=wl_sb[:], in_=wl[:])
            wr_sb = con.tile([128, F_out], dt.bfloat16)
            nc.sync.dma_start(out=wr_sb[:], in_=wr[:])
            if split_P is not None:
                wrx_sb = con.tile([128, F_out], dt.bfloat16)
                nc.sync.dma_start(out=wrx_sb[:], in_=wrx_in[:])
                sinv_sb = con.tile([128, F_out], dt.float32)
                nc.sync.dma_start(out=sinv_sb[:], in_=sinv_in[:])
            else:
                wrx_sb = wr_sb
                att_sb = con.tile([128, F_out], dt.bfloat16)
                nc.sync.dma_start(out=att_sb[:], in_=att_in[:])
            bias_sb = con.tile([128, F_out], dt.float32)
            nc.sync.dma_start(out=bias_sb[:], in_=bias_in[:])
            g_sb = con.tile([128, F_out], dt.float32)
            nc.sync.dma_start(out=g_sb[:], in_=g_in[:])
            b_sb = con.tile([128, F_out], dt.float32)
            nc.sync.dma_start(out=b_sb[:], in_=b_in[:])
            mask_sb = con.tile([128, NB], dt.float32)
            nc.sync.dma_start(out=mask_sb[:], in_=mask_in[:])
            sal = con.tile([128, NPAIR, 2, H + F_out], dt.float32)
            xr_sb = con.tile([128, NB, F_out], dt.float32)
            if dev_q:
                dcol_sb = con.tile([128, C_total], dt.bfloat16)
                nc.sync.dma_start(out=dcol_sb[:], in_=dcol_d[:])
                iota_row = con.tile([128, 128], dt.int32)
                nc.gpsimd.iota(iota_row[:], pattern=[[1, 128]], base=0,
                               channel_multiplier=0)
                iota_row_b = con.tile([128, 128], dt.bfloat16)
                nc.vector.tensor_copy(iota_row_b[:], iota_row[:])
                iota_bc1 = iota_row_b[:].rearrange("p (o f) -> p o f", o=1)

            # xr = x_slice @ Wr, masked to 0 for edge-less dst rows; then
            # xr_sb <- bias - xr so the end-pass needs one add, not two ops.
            ctx = nc.named_scope("xr"); ctx.__enter__()
            XB = G  # reuse the edge-phase PSUM tag/shape
            for t0 in range(0, NB, XB):
                n = min(XB, NB - t0)
                xs_t = st.tile([128, XB, 128], dt.bfloat16, tag="xs")
                nc.sync.dma_start(out=xs_t[:, :n, :],
                                  in_=xTs[:, t0 * 128:(t0 + n) * 128])
                pd = ps_u.tile([128, G, F_out], dt.float32, tag="ups")
                for i in range(n):
                    nc.tensor.matmul(pd[:, i, :], xs_t[:, i, :], wrx_sb[:],
                                     start=True, stop=True)
                    nc.scalar.activation(xr_sb[:, t0 + i, :], pd[:, i, :], A.Copy,
                                         scale=mask_sb[:, t0 + i:t0 + i + 1])
            nc.vector.scalar_tensor_tensor(
                xr_sb[:], xr_sb[:], -1.0,
                bias_sb[:].rearrange("p (o f) -> p o f", o=1)
                .to_broadcast([128, NB, F_out]),
                op0=Op.mult, op1=Op.add)
            ctx.__exit__(None, None, None)

            ctx = nc.named_scope("edge"); ctx.__enter__()
            if split_P is None:
                att_bc1 = att_sb[:].rearrange("p (o f) -> p o f", o=1)
            seg_pair_off = np.concatenate([[0], np.cumsum(SEG_PAIRS)])

            # deferred emission of scatter matmuls + pair drains: keeps the
            # in-order PE queue LAG groups ahead of the eav dependency
            fifo = []

            def _emit(item):
                if item[0] == "so":
                    for ps_ap, q_ap, eav_ap, st_, sp_ in item[1]:
                        nc.tensor.matmul(ps_ap, q_ap, eav_ap, start=st_, stop=sp_)
                else:
                    pair_, so_tile = item[1]
                    nc.scalar.activation(sal[:, pair_, :, :], so_tile[:], A.Copy)

            def _push(item):
                fifo.append(item)
                n_so = sum(1 for it in fifo if it[0] == "so")
                while n_so > LAG:
                    it = fifo.pop(0)
                    _emit(it)
                    if it[0] == "so":
                        n_so -= 1

            def _flush():
                while fifo:
                    _emit(fifo.pop(0))

            for seg in range(len(SEG_PAIRS)):
                for pair in range(seg_pair_off[seg], seg_pair_off[seg + 1]):
                    so_ps = ps_acc.tile([128, 2, H + F_out], dt.float32, tag="sops")
                    for jb in range(2):
                        b = 2 * pair + jb
                        cbk = int(cblk[b])
                        c0 = int(offC[b]) * BLK
                        xts_t = st.tile([128, cbk, 128], dt.bfloat16, tag="xts")
                        nc.sync.dma_start(out=xts_t[:], in_=xts_d[:, c0:c0 + cbk * BLK])
                        xtd_t = st.tile([128, cbk, 128], dt.bfloat16, tag="xtd")
                        nc.sync.dma_start(out=xtd_t[:], in_=xtd_d[:, c0:c0 + cbk * BLK])
                        q_t = st.tile([128, cbk, 128], dt.bfloat16, tag="qt")
                        if dev_q and b % 2 == 1:
                            cc0 = int(offC[b])
                            nc.vector.tensor_tensor(
                                q_t[:],
                                iota_bc1.to_broadcast([128, cbk, 128]),
                                dcol_sb[:, cc0:cc0 + cbk]
                                .rearrange("p (k o) -> p k o", o=1)
                                .to_broadcast([128, cbk, 128]),
                                op=Op.is_equal)
                        else:
                            nc.sync.dma_start(out=q_t[:], in_=q_d[:, c0:c0 + cbk * BLK])

                        for k0 in range(0, cbk, G):
                            g = min(G, cbk - k0)
                            u_ps = ps_u.tile([128, G, F_out], dt.float32, tag="ups")
                            for j in range(g):
                                k = k0 + j
                                nc.tensor.matmul(u_ps[:, j, :], xts_t[:, k, :],
                                                 wl_sb[:], start=True, stop=False)
                                nc.tensor.matmul(u_ps[:, j, :], xtd_t[:, k, :],
                                                 wr_sb[:], start=False, stop=True)
                            lr = ck.tile([128, G, F_out], dt.bfloat16, tag="lr")
                            if split_P is not None:
                                if split_P > 0:
                                    nc.scalar.activation(
                                        lr[:, :g, 0:split_P],
                                        u_ps[:, :g, 0:split_P],
                                        A.Prelu, alpha=NEG_SLOPE)
                                if split_P < F_out:
                                    nc.scalar.activation(
                                        lr[:, :g, split_P:],
                                        u_ps[:, :g, split_P:],
                                        A.Prelu, alpha=1.0 / NEG_SLOPE)
                                amul = lr
                            else:
                                nc.scalar.activation(lr[:, :g, :], u_ps[:, :g, :],
                                                     A.Prelu, alpha=NEG_SLOPE)
                                amul = ck.tile([128, G, F_out], dt.bfloat16, tag="amul")
                                nc.gpsimd.tensor_tensor(
                                    amul[:, :g, :], lr[:, :g, :],
                                    att_bc1.to_broadcast([128, g, F_out]), op=Op.mult)
                            a4 = ck.tile([128, G, H], dt.float32, tag="a4")
                            nc.vector.tensor_reduce(
                                a4[:, :g, :],
                                amul[:, :g, :].rearrange("p g (h c) -> p g h c", h=H),
                                axis=mybir.AxisListType.X, op=Op.add)
                            eav = ck.tile([128, G, H + F_out], dt.bfloat16, tag="eav")
                            nc.scalar.activation(eav[:, :g, 0:H], a4[:, :g, :], A.Exp)
                            nc.vector.tensor_tensor(
                                eav[:, :g, H:].rearrange("p g (h c) -> p g h c", h=H),
                                u_ps[:, :g, :].rearrange("p g (h c) -> p g h c", h=H),
                                eav[:, :g, 0:H].rearrange("p g (h o) -> p g h o", o=1)
                                .to_broadcast([128, g, H, C]),
                                op=Op.mult)
                            if debug and b == 0 and k0 == 0:
                                _flush()
                                du = ck.tile([128, G, F_out], dt.float32, tag="du")
                                nc.vector.tensor_copy(du[:, :g, :], u_ps[:, :g, :])
                                nc.sync.dma_start(
                                    out=dbg_u[:, :g * F_out],
                                    in_=du[:, :g, :].rearrange("p g f -> p (g f)"))
                                de = ck.tile([128, G, H + F_out], dt.float32, tag="de")
                                nc.vector.tensor_copy(de[:, :g, :], eav[:, :g, :])
                                nc.sync.dma_start(
                                    out=dbg_eav[:, :g * (H + F_out)],
                                    in_=de[:, :g, :].rearrange("p g f -> p (g f)"))
                                da = ck.tile([128, G, F_out], dt.float32, tag="da")
                                nc.vector.tensor_copy(da[:, :g, :], amul[:, :g, :])
                                nc.sync.dma_start(
                                    out=dbg_amul[:, :g * F_out],
                                    in_=da[:, :g, :].rearrange("p g f -> p (g f)"))
                            _push(("so", [
                                (so_ps[:, jb, :], q_t[:, k0 + j, :], eav[:, j, :],
                                 k0 + j == 0, k0 + j == cbk - 1)
                                for j in range(g)]))
                    # drain pair accumulators to SBUF (deferred, after last so)
                    _push(("drain", (pair, so_ps)))

                _flush()
                # ---- end-pass for this segment: normalize + LN (+ELU)
                p0, p1 = int(seg_pair_off[seg]), int(seg_pair_off[seg + 1])
                P2 = 2 * (p1 - p0)
                b0 = 2 * p0
                s_v = sal[:, p0:p1, :, 0:H].rearrange("p a two h -> p (a two) h")
                num_v = sal[:, p0:p1, :, H:].rearrange(
                    "p a two (h c) -> p (a two) h c", h=H)
                inv = ep.tile([128, P2, H], dt.float32, tag="inv")
                nc.vector.tensor_scalar(inv[:], s_v, 1e-16, None, op0=Op.add)
                nc.vector.reciprocal(inv[:], inv[:])
                h_t = ep.tile([128, P2, F_out], dt.float32, tag="h")
                nc.vector.tensor_tensor(
                    h_t[:].rearrange("p B (h c) -> p B h c", h=H),
                    num_v,
                    inv[:].rearrange("p B (h o) -> p B h o", o=1)
                    .to_broadcast([128, P2, H, C]),
                    op=Op.mult)
                if split_P is not None:
                    nc.gpsimd.tensor_tensor(
                        h_t[:], h_t[:],
                        sinv_sb[:].rearrange("p (o f) -> p o f", o=1)
                        .to_broadcast([128, P2, F_out]), op=Op.mult)
                nc.vector.tensor_tensor(h_t[:], h_t[:], xr_sb[:, b0:b0 + P2, :],
                                        op=Op.add)
                mu = ep.tile([128, P2, 1], dt.float32, tag="mu")
                nc.vector.tensor_reduce(mu[:], h_t[:], axis=mybir.AxisListType.X,
                                        op=Op.add)
                nc.vector.tensor_scalar(mu[:], mu[:], 1.0 / F_out, None, op0=Op.mult)
                xc = ep.tile([128, P2, F_out], dt.float32, tag="xc")
                nc.vector.tensor_tensor(xc[:], h_t[:],
                                        mu[:].to_broadcast([128, P2, F_out]),
                                        op=Op.subtract)
                sq = ep.tile([128, P2, F_out], dt.float32, tag="sq")
                nc.gpsimd.tensor_tensor(sq[:], xc[:], xc[:], op=Op.mult)
                var = ep.tile([128, P2, 1], dt.float32, tag="var")
                nc.vector.tensor_reduce(var[:], sq[:], axis=mybir.AxisListType.X,
                                        op=Op.add)
                nc.vector.tensor_scalar(var[:], var[:], 1.0 / F_out, LN_EPS,
                                        op0=Op.mult, op1=Op.add)
                rstd = ep.tile([128, P2, 1], dt.float32, tag="rstd")
                nc.vector.reciprocal(rstd[:], var[:])
                nc.scalar.activation(rstd[:], rstd[:], A.Sqrt)
                nc.vector.tensor_tensor(xc[:], xc[:],
                                        rstd[:].to_broadcast([128, P2, F_out]),
                                        op=Op.mult)
                nc.gpsimd.tensor_tensor(
                    xc[:], xc[:],
                    g_sb[:].rearrange("p (o f) -> p o f", o=1)
                    .to_broadcast([128, P2, F_out]), op=Op.mult)
                nc.gpsimd.tensor_tensor(
                    xc[:], xc[:],
                    b_sb[:].rearrange("p (o f) -> p o f", o=1)
                    .to_broadcast([128, P2, F_out]), op=Op.add)
                if layer == 1:
                    m0 = ep.tile([128, P2, F_out], dt.float32, tag="sq")
                    nc.vector.tensor_scalar(m0[:], xc[:], 0.0, None, op0=Op.min)
                    ex = ep.tile([128, P2, F_out], dt.float32, tag="h")
                    nc.scalar.activation(ex[:], m0[:], A.Exp)
                    nc.vector.scalar_tensor_tensor(xc[:], ex[:], -1.0, xc[:],
                                                   op0=Op.add, op1=Op.max)
                nc.sync.dma_start(
                    out=hout[b0 * BLK:(b0 + P2) * BLK, :]
                    .rearrange("(B p) f -> p B f", p=128),
                    in_=xc[:])
            if debug:
                nc.sync.dma_start(
                    out=dbg_xr[:],
                    in_=xr_sb[:].rearrange("p B f -> p (B f)"))
                nc.sync.dma_start(
                    out=dbg_sal[:],
                    in_=sal[:].rearrange("p a two f -> p (a two f)"))
            ctx.__exit__(None, None, None)
    nc.compile()
    return nc


def _fold_att(att, F_out):
    """Column permutation + scales folding single-head attention into the
    weights: pos-att columns first (alpha 0.2), neg-att columns (alpha 5,
    with the extra 0.2 folded into the scale).  Returns (perm, s, P)."""
    att = np.asarray(att, np.float32).reshape(-1)
    perm = np.argsort(att < 0, kind="stable")
    P = int((att >= 0).sum())
    s = np.where(att >= 0, att, NEG_SLOPE * att)[perm]
    s = np.where(s == 0.0, 1e-20, s)  # guard 0*inf for exactly-zero att
    return perm, s, P


def _make_in_maps(meta, x_full, W_l, W_r, att, bias, g_ln, b_ln, F_out,
                  dev_q=False, fold=None):
    def rep(v):
        return np.tile(np.asarray(v, np.float32).reshape(1, F_out), (128, 1))

    wl_b = np.asarray(W_l, np.float32)
    wr_b = np.asarray(W_r, np.float32)
    bias_v, g_v, b_v = bias, g_ln, b_ln
    extra = {}
    if fold is not None:
        perm, s, P = fold
        wrx = wr_b[:, perm].astype(ml_dtypes.bfloat16)
        wl_b = wl_b[:, perm] * s.reshape(1, -1)
        wr_b = wr_b[:, perm] * s.reshape(1, -1)
        bias_v = np.asarray(bias, np.float32)[perm]
        g_v = np.asarray(g_ln, np.float32)[perm]
        b_v = np.asarray(b_ln, np.float32)[perm]
        extra = {"wrx": wrx, "sinv": rep(1.0 / s)}
    else:
        extra = {"att": rep(att).astype(ml_dtypes.bfloat16)}
    wl_b = wl_b.astype(ml_dtypes.bfloat16)
    wr_b = wr_b.astype(ml_dtypes.bfloat16)
    xb = np.asarray(x_full, np.float32).astype(ml_dtypes.bfloat16)
    maps = []
    for c in range(N_CORES):
        core = meta["cores"][c]
        xts, xtd = _perm_streams(meta, xb, core)
        m = {
            "xts": xts, "xtd": xtd,
            "xTs": _slice_stream(x_full, c),
            "wl": wl_b, "wr": wr_b, "bias": rep(bias_v),
            "g": rep(g_v), "b": rep(b_v), "mask": core["mask"],
            **extra,
        }
        m["q"] = core["q"]
        if dev_q:
            m["dcol"] = core["dcol"]
        maps.append(m)
    return maps


def _maybe_install_ntff_hook():
    try:
        import types
        import antenv
        if "antenv.axon_hooks" in sys.modules:
            return True
        mod = types.ModuleType("antenv.axon_hooks")
        state = {"hook": None}
        mod.set_axon_ntff_profile_hook = lambda h: state.__setitem__("hook", h)
        mod.get_axon_ntff_profile_hook = lambda: state["hook"]
        sys.modules["antenv.axon_hooks"] = mod
        antenv.axon_hooks = mod
        from trn_agent_boot.trn_boot import _ntff_profile_via_ctypes
        mod.set_axon_ntff_profile_hook(
            _ntff_profile_via_ctypes("/opt/axon/libaxon_pjrt.so"))
        return True
    except Exception:
        return False


def _run_with_retry(nc, maps, core_ids, trace, tries=3):
    last = None
    for i in range(tries):
        try:
            return run_bass_kernel_spmd(nc, maps, core_ids, trace=trace)
        except Exception as e:  # device flake: retry (fresh exec usually recovers)
            last = e
            if i == tries - 1:
                raise
    raise last


def kernel(**inputs):
    global LAST_EXEC_NS
    LAST_EXEC_NS = {}
    trace = os.environ.get("GAT_TRACE", "0") == "1"
    if trace:
        trace = _maybe_install_ntff_hook()

    x = np.asarray(inputs["x"], np.float32)
    edge_index = np.asarray(inputs["edge_index"])
    meta = _host_prep(edge_index)
    core_ids = list(range(N_CORES))
    debug = os.environ.get("GAT_DEBUG", "0") == "1"

    # ---- layer 1
    nc1 = _build_layer(meta, F_OUT1, H1, layer=1, debug=debug)
    maps1 = _make_in_maps(meta, x, inputs["Wl1"], inputs["Wr1"],
                          np.asarray(inputs["att1"], np.float32).reshape(-1),
                          inputs["bias1"], inputs["g1"], inputs["b1"], F_OUT1)
    res1 = _run_with_retry(nc1, maps1, core_ids, trace)
    h1 = np.concatenate([res1.results[c]["hout"][:S] for c in range(N_CORES)],
                        axis=0)
    if trace:
        LAST_EXEC_NS["layer1"] = res1.exec_time_ns
    if debug:
        np.savez("/root/problem/work/dbg_l1.npz",
                 **{k: np.asarray(res1.results[0][k]) for k in
                    ("dbg_u", "dbg_eav", "dbg_xr", "dbg_sal", "dbg_amul", "hout")})
    if os.environ.get("GAT_L1_ONLY", "0") == "1":
        np.save("/root/problem/work/h1_hw.npy", h1)
        return np.zeros((N, F_OUT2), np.float32)

    # ---- layer 2 (attention folded into the weights, single head)
    att2 = np.asarray(inputs["att2"], np.float32).reshape(-1)
    fold = _fold_att(att2, F_OUT2)
    nc2 = _build_layer(meta, F_OUT2, 1, layer=2, debug=debug, split_P=fold[2])
    maps2 = _make_in_maps(meta, h1, inputs["Wl2"], inputs["Wr2"], att2,
                          inputs["bias2"], inputs["g2"], inputs["b2"], F_OUT2,
                          dev_q=True, fold=fold)
    res2 = _run_with_retry(nc2, maps2, core_ids, trace)
    out_p = np.concatenate([res2.results[c]["hout"][:S] for c in range(N_CORES)],
                           axis=0)
    out = np.empty_like(out_p)
    out[:, fold[0]] = out_p  # undo the column permutation
    if trace:
        LAST_EXEC_NS["layer2"] = res2.exec_time_ns
    if debug:
        np.savez("/root/problem/work/dbg_l2.npz",
                 h1=h1,
                 **{k: np.asarray(res2.results[0][k]) for k in
                    ("dbg_u", "dbg_eav", "dbg_xr", "dbg_sal", "dbg_amul", "hout")})
    return out.astype(np.float32)
